# revision 18
# baseline (speedup 1.0000x reference)
"""Batch x head parallel causal attention block for 8 Trainium2 NeuronCores.

Sharding: 2-way data parallel over batch x 4-way tensor parallel over heads
(per the sharding hint).  Core c handles batch c//4 and heads
[4*(c%4), 4*(c%4)+4): it computes q/k/v projections for its head slice
(columns of wq/wk/wv), RoPE, causal attention, and a row-slice of the
output projection (rows of wo), producing a partial batch output; the host
sums the 4 partials per batch.  Versus pure 8-way head parallel this
halves the per-core x input (8.4MB) and partial output (8.4MB): the
output-projection phase produces 8.4MB over ~55us of matmuls (~150GB/s),
so the final HBM-write drain that dominated the old kernel's tail
disappears, and there is a single A->B->C phase sequence instead of two.

All matmuls run in bf16 (1 cycle/row on the PE).  fp8 was evaluated and
rejected: e4m3 noise on any projection pushes max-err past the 2e-2 gate
(measured 4-5e-2 in simulation).  Scores are computed transposed
(S^T[k, q]) so the softmax renormalization folds into PE ones-matmuls and
P needs no transposes before P@V; off-diagonal P pairs are pre-added on
the idle DVE so one ones-matmul per pair replaces two.  The RoPE
rotate-half partition swap runs as two SBUF->SBUF copies issued from the
ACT queue.  V is projected transposed and moved to natural layout by PE
transposes sharing one PSUM bank pair.  Diagonal causal blocks are
q-sliced so fully-masked regions are never computed.  Projection PSUM
tiles rotate through a 6-slot pool so each drain lands under later
blocks' matmuls.  Output partials are written bf16 and summed on the
host in fp32.
"""

import math
import sys

sys.path.insert(0, "/opt/trn_rl_repo")

import numpy as np
import ml_dtypes

B = 2
S = 2048
E = 2048
H = 16
D = 128
ROPE_BASE = 10000.0
NCORES = 8
HPC = 4                    # heads per core
DC = HPC * D               # head-dim cols per core = 512
KC = E // 128              # 16 contraction chunks
TC8 = 512                  # token chunk for projections
NTC8 = S // TC8            # 4
SB512 = 512                # query super-block
NSB = S // SB512           # 4
SCALE = 1.0 / math.sqrt(D)

_COMPILED = None


def _build_program():
    import concourse.bass as bass
    import concourse.mybir as mybir
    from concourse import bacc
    from concourse.tile import TileContext

    f32 = mybir.dt.float32
    bf16 = mybir.dt.bfloat16

    def fr(ap):
        return ap.bitcast(mybir.dt.float32r)

    nc = bacc.Bacc()
    # host-blocked layouts: every DMA moves a contiguous per-partition span
    xT_d = nc.declare_dram_parameter("xT", [NTC8, 128, KC, TC8], bf16, isOutput=False)
    cos_d = nc.declare_dram_parameter("cosF", [128, S], bf16, isOutput=False)
    sin_d = nc.declare_dram_parameter("sinF", [128, S], bf16, isOutput=False)
    wq_d = nc.declare_dram_parameter("wq", [128, KC, DC], bf16, isOutput=False)
    wk_d = nc.declare_dram_parameter("wk", [128, KC, DC], bf16, isOutput=False)
    wv_d = nc.declare_dram_parameter("wv", [128, KC, DC], bf16, isOutput=False)
    wo_d = nc.declare_dram_parameter("wo", [128, HPC, E], bf16, isOutput=False)
    on_d = nc.declare_dram_parameter("ones", [128, 128], bf16, isOutput=False)
    id_d = nc.declare_dram_parameter("ident", [128, 128], f32, isOutput=False)
    out_d = nc.declare_dram_parameter("out", [S // 128, 128, E], bf16, isOutput=True)

    Exp = mybir.ActivationFunctionType.Exp
    mult = mybir.AluOpType.mult
    add = mybir.AluOpType.add

    with TileContext(nc) as tc:
        with (
            tc.tile_pool(name="wpool", bufs=1) as wp,
            tc.tile_pool(name="persist", bufs=1) as pp,
            tc.tile_pool(name="xin", bufs=8) as xp,
            tc.tile_pool(name="rope", bufs=3) as rp,
            tc.tile_pool(name="ptile", bufs=6) as ptp,
            tc.tile_pool(name="small", bufs=2) as smp,
            tc.tile_pool(name="outsb", bufs=3) as op,
        ):
            # ---- resident weights / constants ----
            wq_sb = wp.tile([128, KC, DC], bf16)
            wk_sb = wp.tile([128, KC, DC], bf16)
            wv_sb = wp.tile([128, KC, DC], bf16)
            wo_sb = wp.tile([128, HPC, E], bf16)
            cos_sb = wp.tile([128, S], bf16)
            sin_sb = wp.tile([128, S], bf16)
            ones_sb = wp.tile([128, 128], bf16)
            ident_sb = wp.tile([128, 128], f32)

            # ---- persistent arrays ----
            qT = [pp.tile([128, S], bf16, name=f"qT{h}", tag=f"qT{h}") for h in range(HPC)]
            kT = [pp.tile([128, S], bf16, name=f"kT{h}", tag=f"kT{h}") for h in range(HPC)]
            v_sb = pp.tile([128, S // 128, DC], bf16, name="v_sb", tag="v")
            zn = [pp.tile([128, S], bf16, name=f"zn{h}", tag=f"zn{h}") for h in range(HPC)]

            # x DMAs emitted up-front; the xin pool has 7 slots so the sync
            # queue stalls benignly on slot reuse while the consts/weights
            # ride the ACT/POOL queues in parallel
            xq_tiles = []

            def emit_x_dmas():
                for tc8 in range(NTC8):
                    for qtr in range(4):
                        xq = xp.tile([128, 4, TC8], bf16, name="xq", tag="xq")
                        if tc8 == 0 and qtr == 0:
                            # first chunk split small and wide across queues so
                            # the very first matmul starts as early as possible
                            nc.sync.dma_start(out=xq[:, 0:1, 0:TC8 // 2], in_=xT_d[0, :, 0:1, 0:TC8 // 2])
                            nc.gpsimd.dma_start(out=xq[:, 0:1, TC8 // 2:TC8], in_=xT_d[0, :, 0:1, TC8 // 2:TC8])
                            nc.scalar.dma_start(out=wq_sb[:, 0:2, :], in_=wq_d[:, 0:2, :])
                            nc.gpsimd.dma_start(out=wq_sb[:, 2:4, :], in_=wq_d[:, 2:4, :])
                            nc.sync.dma_start(out=xq[:, 1:4, :], in_=xT_d[0, :, 1:4, :])
                            nc.scalar.dma_start(out=wq_sb[:, 4:KC, :], in_=wq_d[:, 4:KC, :])
                        else:
                            nc.sync.dma_start(
                                out=xq[:], in_=xT_d[tc8, :, qtr * 4:(qtr + 1) * 4, :])
                        xq_tiles.append(xq)
                    if tc8 == 0:
                        # halves of wk/wv stream on both spare queues so each
                        # lands just ahead of its first consumer block
                        nc.scalar.dma_start(out=wk_sb[:, 0:8, :], in_=wk_d[:, 0:8, :])
                        nc.gpsimd.dma_start(out=wk_sb[:, 8:KC, :], in_=wk_d[:, 8:KC, :])
                        nc.gpsimd.dma_start(out=cos_sb[:], in_=cos_d[:])
                        nc.gpsimd.dma_start(out=sin_sb[:], in_=sin_d[:])
                    if tc8 == 1:
                        nc.scalar.dma_start(out=wv_sb[:, 0:8, :], in_=wv_d[:, 0:8, :])
                        nc.gpsimd.dma_start(out=wv_sb[:, 8:KC, :], in_=wv_d[:, 8:KC, :])
                        nc.gpsimd.dma_start(out=fr(ident_sb[:]), in_=fr(id_d[:]))
                        nc.gpsimd.dma_start(out=ones_sb[:], in_=on_d[:])
                    if tc8 == 2:
                        nc.scalar.dma_start(out=wo_sb[:], in_=wo_d[:])

            # ============ Phase A: projections + RoPE + V transpose ============
            with (
                tc.tile_pool(name="psA", bufs=6, space="PSUM") as pA,
                tc.tile_pool(name="psR", bufs=2, space="PSUM") as pR,
            ):
                emit_x_dmas()
                for tc8 in range(NTC8):
                    s0 = tc8 * TC8
                    xqs = xq_tiles[tc8 * 4:(tc8 + 1) * 4]

                    def xts(kc):
                        return xqs[kc // 4][:, kc % 4, :]

                    # 12 projection blocks per chunk rotate through 6 PSUM
                    # slots; each block's drain (ACT copy + DVE RoPE or PE
                    # transpose) lands under later blocks' matmuls.
                    qk_seq = [(wq_sb, h, qT[h]) for h in range(HPC)] + \
                             [(wk_sb, h, kT[h]) for h in range(HPC)]
                    pending = []   # (tmp, dst) waiting for swap+DVE emission

                    def emit_rope_tail(tmp, dst):
                        # rotate-half via two SBUF->SBUF half-swap copies
                        # issued from the ACT queue
                        rot = rp.tile([128, TC8], bf16, name="rot", tag="rot")
                        nc.scalar.dma_start(out=rot[0:64, :], in_=tmp[64:128, :])
                        nc.scalar.dma_start(out=rot[64:128, :], in_=tmp[0:64, :])
                        nc.vector.tensor_tensor(tmp[:], tmp[:], cos_sb[:, s0:s0 + TC8], mult)
                        nc.vector.tensor_tensor(rot[:], rot[:], sin_sb[:, s0:s0 + TC8], mult)
                        nc.vector.tensor_tensor(dst[:, s0:s0 + TC8], tmp[:], rot[:], add)

                    def emit_qk_block(w_sb, h, dst):
                        ps = pA.tile([128, TC8], f32, name="proj_ps", tag="proj")
                        for kc in range(KC):
                            nc.tensor.matmul(ps[:], lhsT=w_sb[:, kc, h * D:(h + 1) * D],
                                             rhs=xts(kc), start=(kc == 0), stop=(kc == KC - 1))
                        if pending:
                            emit_rope_tail(*pending.pop())
                        tmp = rp.tile([128, TC8], bf16, name="tmp", tag="tmp", bufs=5)
                        nc.scalar.copy(tmp[:], ps[:])
                        pending.append((tmp, dst))

                    def emit_v_block(h):
                        # v blocks: transposed projection, drained via ACT
                        # copy + PE transposes into natural layout.
                        ps = pA.tile([128, TC8], f32, name="proj_ps", tag="proj")
                        for kc in range(KC):
                            nc.tensor.matmul(ps[:], lhsT=wv_sb[:, kc, h * D:(h + 1) * D],
                                             rhs=xts(kc), start=(kc == 0), stop=(kc == KC - 1))
                        if pending:
                            emit_rope_tail(*pending.pop())
                        vt = rp.tile([128, TC8], f32, name="vt", tag="vt", bufs=2)
                        nc.scalar.copy(fr(vt[:]), ps[:])
                        tp = pR.tile([128, 4, 128], f32, name="tp", tag="tp")
                        for tb in range(TC8 // 128):
                            nc.tensor.matmul(fr(tp[:, tb, :]), lhsT=fr(vt[:, tb * 128:(tb + 1) * 128]),
                                             rhs=fr(ident_sb[:]), is_transpose=True,
                                             skip_group_check=True)
                        nc.vector.tensor_copy(v_sb[:, s0 // 128:s0 // 128 + 4, h * D:(h + 1) * D],
                                              tp[:, 0:4, :])

                    if tc8 == 0:
                        # kc-lockstep over the 4 q blocks: each freshly-landed
                        # weight chunk feeds 4 matmuls, so the first blocks
                        # need weight bandwidth at 1/4 the usual rate and the
                        # PE starts as soon as wq[kc=0] and half a quarter of
                        # x have arrived
                        qps = [pA.tile([128, TC8], f32, name="proj_ps", tag="proj") for _ in range(HPC)]
                        for kc in range(KC):
                            for h in range(HPC):
                                nc.tensor.matmul(qps[h][:], lhsT=wq_sb[:, kc, h * D:(h + 1) * D],
                                                 rhs=xts(kc), start=(kc == 0), stop=(kc == KC - 1))
                        for h in range(HPC):
                            tmp = rp.tile([128, TC8], bf16, name="tmp", tag="tmp", bufs=5)
                            nc.scalar.copy(tmp[:], qps[h][:])
                            pending.append((tmp, qT[h]))
                        for args in qk_seq[HPC:]:
                            emit_qk_block(*args)
                        for h in range(HPC):
                            emit_v_block(h)
                    elif tc8 < NTC8 - 1:
                        for args in qk_seq:
                            emit_qk_block(*args)
                        for h in range(HPC):
                            emit_v_block(h)
                    else:
                        # last chunk: V first so its transpose chain drains
                        # under the q/k matmuls and the A->B transition only
                        # waits on the final rope tail
                        for h in range(HPC):
                            emit_v_block(h)
                        for args in qk_seq:
                            emit_qk_block(*args)
                    if pending:
                        emit_rope_tail(*pending.pop())

            # ============ Phase B: causal attention ============
            with tc.tile_pool(name="psB", bufs=1, space="PSUM") as pB:
                for sb in range(NSB):
                    for h in range(HPC):
                        nkb = (sb + 1) * (SB512 // 128)
                        ndiag0 = sb * (SB512 // 128)
                        pts = [None] * nkb
                        pairsum = {}
                        first_sum = [True]
                        # off-diagonal blocks process as PAIRS sharing one
                        # two-bank PSUM tile and a SINGLE exp over [128,2,512]
                        # (halves the ACT instruction count in the ACT-bound
                        # attention phase); diagonal blocks stay singles
                        units = [("pair", k) for k in range(0, ndiag0, 2)] + \
                                [("diag", ndiag0 + d) for d in range(SB512 // 128)]

                        def emit_score_unit(u):
                            kind, k0_ = u
                            st2 = pB.tile([128, 2, SB512], f32, name="st2", tag="st", bufs=2)
                            pt2 = ptp.tile([128, 2, SB512], bf16, name="pt2", tag="pt", bufs=4)
                            if kind == "pair":
                                for i in range(2):
                                    nc.tensor.matmul(st2[:, i, :],
                                                     lhsT=kT[h][:, (k0_ + i) * 128:(k0_ + i + 1) * 128],
                                                     rhs=qT[h][:, sb * SB512:(sb + 1) * SB512],
                                                     start=True, stop=True)
                                nc.scalar.activation(pt2[:], st2[:], Exp, scale=SCALE)
                                ps2 = ptp.tile([128, SB512], bf16, name="ptsum", tag="ptsum", bufs=3)
                                nc.vector.tensor_tensor(ps2[:], pt2[:, 0, :], pt2[:, 1, :], add)
                                pairsum[k0_ + 1] = ps2
                                pts[k0_] = (pt2, 0, 0, SB512)
                                pts[k0_ + 1] = (pt2, 1, 0, SB512)
                            else:
                                delta = k0_ - ndiag0
                                q0 = 128 * delta
                                W = SB512 - q0
                                nc.tensor.matmul(st2[:, 0, :W],
                                                 lhsT=kT[h][:, k0_ * 128:(k0_ + 1) * 128],
                                                 rhs=qT[h][:, sb * SB512 + q0:(sb + 1) * SB512],
                                                 start=True, stop=True)
                                nc.scalar.activation(pt2[:, 0, :W], st2[:, 0, :W], Exp, scale=SCALE)
                                nc.gpsimd.affine_select(
                                    out=pt2[:, 0, :W], in_=pt2[:, 0, :W],
                                    pattern=[[1, W]], compare_op=mybir.AluOpType.is_ge,
                                    fill=0.0, base=0, channel_multiplier=-1,
                                )
                                pts[k0_] = (pt2, 0, q0, W)

                        def emit_zsum(kblk):
                            pt2, plane, q0, W = pts[kblk]
                            delta = kblk - ndiag0
                            nc.tensor.matmul(z_ps[:, q0:SB512], lhsT=v_sb[:, kblk, h * D:(h + 1) * D],
                                             rhs=pt2[:, plane, :W], start=(kblk == 0), stop=(kblk == nkb - 1))
                            if delta < 0:
                                if kblk % 2 == 1:
                                    ps2 = pairsum.pop(kblk)
                                    nc.tensor.matmul(sum_ps[:], lhsT=ones_sb[:], rhs=ps2[:],
                                                     start=first_sum[0], stop=False)
                                    first_sum[0] = False
                            else:
                                nc.tensor.matmul(sum_ps[:, q0:SB512], lhsT=ones_sb[:],
                                                 rhs=pt2[:, plane, :W],
                                                 start=first_sum[0], stop=(kblk == nkb - 1))
                                first_sum[0] = False
                            pts[kblk] = None

                        LOOK = 2   # score units emitted ahead of z/sum
                        for ui in range(min(LOOK, len(units))):
                            emit_score_unit(units[ui])
                        z_ps = pB.tile([128, SB512], f32, name="z_ps", tag="z", bufs=2)
                        sum_ps = pB.tile([128, SB512], f32, name="sum_ps", tag="sum", bufs=2)
                        kblk = 0
                        for ui, u in enumerate(units):
                            if ui + LOOK < len(units):
                                emit_score_unit(units[ui + LOOK])
                            nblk = 2 if u[0] == "pair" else 1
                            for _ in range(nblk):
                                emit_zsum(kblk)
                                kblk += 1
                        rep_sb = smp.tile([128, SB512], f32, name="rep_sb", tag="repsb")
                        nc.vector.reciprocal_approx_fast(out=rep_sb[:], in_=sum_ps[:])
                        nc.vector.tensor_tensor(zn[h][:, sb * SB512:(sb + 1) * SB512],
                                                z_ps[:], rep_sb[:], mult)

            # ============ Phase C: output projection ============
            with tc.tile_pool(name="psC", bufs=4, space="PSUM") as pC:
                for tb in range(S // 128):
                    o_sb = op.tile([128, E], bf16, name="o_sb", tag="osb")
                    for ec in range(E // 512):
                        o_ps = pC.tile([128, 512], f32, name="o_ps", tag="o")
                        for h in range(HPC):
                            nc.tensor.matmul(o_ps[:], lhsT=zn[h][:, tb * 128:(tb + 1) * 128],
                                             rhs=wo_sb[:, h, ec * 512:(ec + 1) * 512],
                                             start=(h == 0), stop=(h == HPC - 1))
                        if ec % 2 == 0:
                            nc.vector.tensor_copy(o_sb[:, ec * 512:(ec + 1) * 512], o_ps[:])
                        else:
                            nc.scalar.copy(o_sb[:, ec * 512:(ec + 1) * 512], o_ps[:])
                    if tb >= S // 128 - 2:
                        # shorten the final drain: last tiles ship in halves
                        nc.sync.dma_start(out=out_d[tb, :, 0:E // 2], in_=o_sb[:, 0:E // 2])
                        nc.sync.dma_start(out=out_d[tb, :, E // 2:E], in_=o_sb[:, E // 2:E])
                    else:
                        nc.sync.dma_start(out=out_d[tb], in_=o_sb[:])

    nc.compile()
    return nc


def _get_compiled():
    global _COMPILED
    if _COMPILED is None:
        _COMPILED = _build_program()
    return _COMPILED


def _host_inputs(x, wq, wk, wv, wo):
    bf = ml_dtypes.bfloat16
    x = np.asarray(x, dtype=np.float32)
    # per-batch xT blocked: [NTC8, 128, KC, TC8];
    # element (tc8, p, kc, c) = x[b, tc8*TC8+c, kc*128+p]
    xTs = [
        np.ascontiguousarray(
            x[b].reshape(NTC8, TC8, KC, 128).transpose(0, 3, 2, 1)
        ).astype(bf)
        for b in range(B)
    ]

    pos = np.arange(S, dtype=np.float32)
    inv_freq = (1.0 / (ROPE_BASE ** (np.arange(0, D, 2, dtype=np.float32) / np.float32(D)))).astype(np.float32)
    ang = pos[:, None] * inv_freq[None, :]          # (S, 64) fp32
    cos_h = np.cos(ang).astype(np.float32)
    sin_h = np.sin(ang).astype(np.float32)
    cosF = np.ascontiguousarray(np.concatenate([cos_h.T, cos_h.T], axis=0)).astype(bf)   # (128, S)
    sinF = np.ascontiguousarray(np.concatenate([-sin_h.T, sin_h.T], axis=0)).astype(bf)  # (128, S)
    ones = np.ones((128, 128), dtype=np.float32).astype(bf)
    ident = np.eye(128, dtype=np.float32)

    wq = np.asarray(wq, dtype=np.float32)
    wk = np.asarray(wk, dtype=np.float32)
    wv = np.asarray(wv, dtype=np.float32)
    wo = np.asarray(wo, dtype=np.float32)

    maps = []
    for c in range(NCORES):
        b = c // (NCORES // B)
        hg = c % (NCORES // B)
        sl = slice(hg * DC, (hg + 1) * DC)
        maps.append({
            "xT": xTs[b],
            "cosF": cosF,
            "sinF": sinF,
            "wq": np.ascontiguousarray(wq[:, sl].reshape(KC, 128, DC).transpose(1, 0, 2)).astype(bf),
            "wk": np.ascontiguousarray(wk[:, sl].reshape(KC, 128, DC).transpose(1, 0, 2)).astype(bf),
            "wv": np.ascontiguousarray(wv[:, sl].reshape(KC, 128, DC).transpose(1, 0, 2)).astype(bf),
            "wo": np.ascontiguousarray(wo[sl, :].reshape(HPC, 128, E).transpose(1, 0, 2)).astype(bf),
            "ones": ones,
            "ident": ident,
        })
    return maps


def kernel(x, wq, wk, wv, wo, _trace=False):
    from concourse.bass_utils import run_bass_kernel_spmd

    nc = _get_compiled()
    maps = _host_inputs(x, wq, wk, wv, wo)
    res = run_bass_kernel_spmd(nc, maps, list(range(NCORES)), trace=_trace)
    gpb = NCORES // B
    out = np.zeros((B, S // 128, 128, E), dtype=np.float32)
    for c in range(NCORES):
        b = c // gpb
        out[b] += res.results[c]["out"].astype(np.float32)
    out = out.reshape(B, S, E)
    if _trace:
        kernel.last_exec_time_ns = res.exec_time_ns
        kernel.last_trace = res.instructions_and_trace
    return out
